# revision 42
# baseline (speedup 1.0000x reference)
"""Trainium2 SPMD kernel for DistanceContrastiveLoss.

Math:
  d2[i,j] = ||c_i||^2 + ||s_j||^2 - 2 c_i.s_j
  sim     = -exp(t) * sqrt(d2)
  loss    = 0.5*(CE(sim, diag) + CE(sim.T, diag))
          = 0.5*( mean_i(LSE_row_i - sim_ii) + mean_j(LSE_col_j - sim_jj) )

Sharding: rows of the 8192x8192 logits matrix are split across 8 cores
(1024 rows each). Per core the block is processed as 32 steps of
[128 x 2048] (row-tile rt 0..7 major, column-group g 0..3 minor).

Engine pipeline (ACT is the critical path at ~2 table ops/element):
  PE  : d2_part = -2 c.s            (2x K=128 bf16 matmuls -> PSUM)
  DVE : ring = d2_part + s2b        (drains PSUM -> SBUF f16; s2b is the
                                     host-prebroadcast ||s_j||^2 row, f16)
  ACT : dist = sqrt(ring + c2_i)    (c2 via per-partition bias; f16 out)
  ACT : w = exp(-a*dist + B)        (8192-wide per row-tile; bf16 out;
                                     accum_out -> row sums, fp32)
  PE  : colpart += ones32.T @ w     (per-row-tile partition reduce,
                                     strip-packed, accumulated in a
                                     persistent 4-bank PSUM tile across
                                     all row-tiles via start/stop flags)

Sqrt and Exp live in different ACT function tables, so the ACT queue is
batched 2 row-tiles at a time ([8x sqrt][table swap][2x exp][table swap])
to amortize the 1.3us table loads. The f16 ring decouples PE from ACT so
the tensor engine keeps producing during exp phases.

The final log/diagonal/mean is a tiny O(N) host epilogue; partial column
sums from the 8 cores are summed on host (cheaper than a collective).
"""

import os

os.environ.setdefault("MYCRO_LOCAL_CACHE", "1")

import numpy as np

import concourse.bacc as bacc
import concourse.bass as bass
import concourse.mybir as mybir
import concourse.tile as tile
from concourse.bass_utils import run_bass_kernel_spmd

F32 = mybir.dt.float32
F16 = mybir.dt.float16
BF16 = mybir.dt.bfloat16
AF = mybir.ActivationFunctionType

# Full-problem geometry (hardcoded per the task contract).
N = 8192
D = 256
NCORES = 8
ROWS_PER_CORE = N // NCORES  # 1024
P = 128  # partitions


def build(n_rt: int = 8, n_groups: int = 4, debug: bool = False, reps: int = 1,
          batch_rt: int = 2, ring_bufs: int = 8, probe: str = ""):
    """Build the SPMD Bass program.

    n_rt      : number of 128-row tiles per core        (full: 8)
    n_groups  : number of 2048-wide column groups       (full: 4)
    batch_rt  : row-tiles per ACT table phase (sqrt/exp batching)
    """
    rows = P * n_rt
    cols = 2048 * n_groups
    n_ct = cols // 512  # 512-wide column chunks (colsum output rows)

    nc = bacc.Bacc(
        "TRN2", target_bir_lowering=False, debug=debug, num_devices=NCORES
    )

    lhsT_d = nc.dram_tensor("lhsT", [D, rows], BF16, kind="ExternalInput")
    rhsT_d = nc.dram_tensor("rhsT", [D, cols], BF16, kind="ExternalInput")
    s2b_d = nc.dram_tensor("s2b", [P, cols], F16, kind="ExternalInput")
    c2_d = nc.dram_tensor("c2m", [P, n_rt], F32, kind="ExternalInput")
    cst_d = nc.dram_tensor("cst", [P, 2], F32, kind="ExternalInput")

    # rowsum slots 0..n_rt-2 are full row-tile sums; the last row-tile's exp
    # is split into 4 column quarters (tail hiding), slots n_rt-1..n_rt+2.
    rowsums_d = nc.dram_tensor("rowsums", [P, n_rt + 3], F32, kind="ExternalOutput")
    # colsums row m holds, for bank b, the partials of columns
    # [2048*b + 512*m, 2048*b + 512*(m+1)) at offset 512*b (strip packing)
    colsums_d = nc.dram_tensor("colsums", [4, 2048], F32, kind="ExternalOutput")

    with tile.TileContext(nc) as tc:
        with (
            tc.tile_pool(name="inp", bufs=1) as inp,
            tc.tile_pool(name="cstp", bufs=1) as cstp,
            tc.tile_pool(name="outp", bufs=1) as outp,
            tc.tile_pool(name="ring", bufs=ring_bufs) as ring,
            tc.tile_pool(name="distp", bufs=2) as distp,
            tc.tile_pool(name="wp", bufs=3) as wp,
            tc.tile_pool(name="d2p", bufs=2, space=bass.MemorySpace.PSUM) as d2p,
            tc.tile_pool(name="colpartp", bufs=1, space=bass.MemorySpace.PSUM) as colpartp,
        ):
            # ---- load inputs (small tensors first: they gate step 0) -------
            cst_sb = inp.tile([P, 2], F32, tag="cst")
            nc.sync.dma_start(out=cst_sb[:], in_=cst_d.ap()[:])
            c2_sb = inp.tile([P, n_rt], F32, tag="c2")
            nc.sync.dma_start(out=c2_sb[:], in_=c2_d.ap()[:])
            lhs_sb = []
            for kk in range(2):
                t = inp.tile([P, rows], BF16, tag=f"lhs{kk}")
                nc.sync.dma_start(out=t[:], in_=lhsT_d.ap()[kk * P : (kk + 1) * P, :])
                lhs_sb.append(t)
            # rhs + s2b in column chunks so the first steps start early
            rhs_sb = []
            for kk in range(2):
                rhs_t = inp.tile([P, cols], BF16, tag=f"rhs{kk}")
                rhs_sb.append(rhs_t)
            s2b_sb = inp.tile([P, cols], F16, tag="s2b")
            # finer chunks up front so the first drains/sqrts unblock early
            bounds = [0, 1024, 2048, 3072, 4096, 6144, 8192]
            for cb, ce in zip(bounds[:-1], bounds[1:]):
                for kk in range(2):
                    nc.sync.dma_start(
                        out=rhs_sb[kk][:, cb:ce],
                        in_=rhsT_d.ap()[kk * P : (kk + 1) * P, cb:ce],
                    )
                nc.sync.dma_start(
                    out=s2b_sb[:, cb:ce],
                    in_=s2b_d.ap()[:, cb:ce],
                )

            ones32 = cstp.tile([P, 32], BF16)  # column-sum stationary operand
            nc.vector.memset(ones32[:], 1.0)

            rowsum_tiles = []
            for rr in range(2):
                rt_t = outp.tile([P, n_rt + 3], F32, tag=f"rowsum{rr}")
                rowsum_tiles.append(rt_t)
            cs_sb = outp.tile([P, 2048], F32)

            neg_a = cst_sb[:, 0:1]
            bias_b = cst_sb[:, 1:2]
            scratch = cstp.tile([P, 1024], F16)  # probe target only

            # ---- main loop ------------------------------------------------
            # 64 substeps u: rt = u//8 row-tile, h = u%8 1024-col chunk.
            # Two substeps share one 2048-wide ring tile / sqrt op.
            n_batches = (n_rt + batch_rt - 1) // batch_rt

            w_tiles = {}
            dist_tiles = {}
            ring_cur = [None]
            rowsum_cur = [rowsum_tiles[0]]

            for _rep in range(reps):
                rowsum = rowsum_tiles[_rep % 2]
                rowsum_cur[0] = rowsum
                final_rep = _rep == reps - 1

                def pe_step(u):
                    rt, h = divmod(u, 8)
                    d2 = d2p.tile([P, 1024], F32, tag="d2", name="d2")
                    for kk in range(2):
                        lh = lhs_sb[kk][:, rt * P : (rt + 1) * P]
                        for j in range(0, 1024, 512):
                            colb = 1024 * h + j
                            nc.tensor.matmul(
                                d2[:, j : j + 512],
                                lh,
                                rhs_sb[kk][:, colb : colb + 512],
                                start=(kk == 0),
                                stop=(kk == 1),
                            )
                    return d2

                def drain(u, d2):
                    rt, h = divmod(u, 8)
                    if h % 2 == 0:
                        ring_cur[0] = ring.tile([P, 2048], F16, tag="rg", name="rg")
                    half = ring_cur[0][:, 1024 * (h % 2) : 1024 * (h % 2) + 1024]
                    nc.vector.tensor_add(
                        half, d2[:], s2b_sb[:, 1024 * h : 1024 * h + 1024]
                    )
                    if probe == "dve" and h % 2 == 0:
                        nc.vector.memset(scratch[:], 1.0)
                    if probe == "pe" and h % 2 == 0:
                        nc.tensor.matmul(
                            d2[:, 0:512],
                            lhs_sb[0][:, rt * P : (rt + 1) * P],
                            rhs_sb[0][:, 0:512],
                            start=True,
                            stop=True,
                        )
                    if probe == "act" and h % 2 == 0:
                        nc.scalar.activation(
                            scratch[:, 0:512],
                            half[:, 0:512],
                            AF.Sqrt,
                            bias=c2_sb[:, rt : rt + 1],
                            scale=1.0,
                        )

                def sqrt_step(u):
                    # one 2048-wide sqrt per substep pair (odd h)
                    rt, h = divmod(u, 8)
                    if rt not in dist_tiles:
                        dist_tiles[rt] = distp.tile(
                            [P, cols], F16, tag="dist", name="dist"
                        )
                    base = 1024 * (h - 1)
                    if _rep == 0 and u == 1:
                        # first pair of the run: split into halves so the
                        # first sqrt starts before the second DMA chunk and
                        # drain have landed
                        for off in (0, 1024):
                            nc.scalar.activation(
                                dist_tiles[rt][:, base + off : base + off + 1024],
                                ring_cur[0][:, off : off + 1024],
                                AF.Sqrt,
                                bias=c2_sb[:, rt : rt + 1],
                                scale=1.0,
                            )
                        return
                    nc.scalar.activation(
                        dist_tiles[rt][:, base : base + 2048],
                        ring_cur[0][:],
                        AF.Sqrt,
                        bias=c2_sb[:, rt : rt + 1],
                        scale=1.0,
                    )

                def exp_step(rt, split=False):
                    w = wp.tile([P, cols], BF16, tag="w", name="w")
                    dist = dist_tiles.pop(rt)
                    rs = rowsum_cur[0]
                    if not split:
                        nc.scalar.activation(
                            w[:],
                            dist[:],
                            AF.Exp,
                            bias=bias_b,
                            scale=neg_a,
                            accum_out=rs[:, rt : rt + 1],
                        )
                    else:
                        # final row-tile of the final rep: 4 column quarters
                        # so the colsum strips (PE) start before the full
                        # exp is done (tail hiding)
                        for q in range(4):
                            nc.scalar.activation(
                                w[:, 2048 * q : 2048 * q + 2048],
                                dist[:, 2048 * q : 2048 * q + 2048],
                                AF.Exp,
                                bias=bias_b,
                                scale=neg_a,
                                accum_out=rs[:, rt + q : rt + q + 1],
                            )
                    w_tiles[rt] = w

                def strips(rt, ks):
                    # partition-reduce w chunks into the strip-packed PSUM
                    # accumulator (chunk k=4b+m -> partitions 32m.., bank b);
                    # regions accumulate across row-tiles via start/stop
                    w = w_tiles[rt]
                    for k in ks:
                        b, m = divmod(k, 4)
                        nc.tensor.matmul(
                            colpart[32 * m : 32 * m + 32, 512 * b : 512 * b + 512],
                            ones32[:, :],
                            w[:, 512 * k : 512 * k + 512],
                            start=(rt == 0),
                            stop=(rt == n_rt - 1),
                            tile_position=(0, 32 * m),
                            skip_group_check=True,
                        )

                def acc_step(rt):
                    strips(rt, range(n_ct))
                    del w_tiles[rt]

                def emit_outputs(rs):
                    nc.vector.tensor_copy(cs_sb[:], colpart[:])
                    for m in range(4):
                        nc.sync.dma_start(
                            out=colsums_d.ap()[m : m + 1, :],
                            in_=cs_sb[32 * m : 32 * m + 1, :],
                        )
                    nc.sync.dma_start(out=rowsums_d.ap()[:], in_=rs[:])

                if _rep == 0:
                    colpart = colpartp.tile([P, 2048], F32, tag="cp", name="cp")

                for c in range(n_batches):
                    rt_lo = c * batch_rt
                    rt_hi = min(rt_lo + batch_rt, n_rt)
                    # pair-major across the batch's row-tiles: early substeps
                    # reuse the first rhs column chunks, so the DMA prologue
                    # stops gating the first sqrts
                    for p in range(4):
                        for rt in range(rt_lo, rt_hi):
                            for h in (2 * p, 2 * p + 1):
                                u = rt * 8 + h
                                d2 = pe_step(u)
                                drain(u, d2)
                                if h % 2 == 1:
                                    sqrt_step(u)
                    # delayed colsum strips: the previous batch's (or, at
                    # c==0, the previous REP's last batch's) strips flow
                    # straight after this batch's step matmuls on a warm PE
                    # without blocking the next batch's drains. The fence
                    # stops the list scheduler hoisting the exp-gated strips
                    # ahead of the step matmuls in the PE queue (that stalls
                    # PE mid exp phase and starves the next sqrt phase).
                    tc.no_sync_barrier()
                    if c == 0 and _rep > 0:
                        for prt in (n_rt - 2, n_rt - 1):
                            acc_step(prt)
                        emit_outputs(rowsum_tiles[(_rep - 1) % 2])
                    elif c > 0:
                        for prt in range(rt_lo - batch_rt, rt_lo):
                            if prt in w_tiles:
                                acc_step(prt)
                    # scheduler-only fences: keep the ACT queue batched as
                    # [8x sqrt][2x exp] so table swaps stay amortized (the
                    # list scheduler otherwise interleaves exp into sqrt
                    # stalls, doubling the table loads). No runtime sync.
                    tc.no_sync_barrier()
                    last_batch = rt_hi == n_rt
                    if final_rep and last_batch:
                        # keep the PE p-state ramped through the final exp
                        # phase so the tail strips run at full clock: dummy
                        # free-running matmuls into the rotating PSUM tiles
                        for i in range(44):
                            dmy = d2p.tile([P, 1024], F32, tag="d2", name="dmy")
                            nc.tensor.matmul(
                                dmy[:, 0:512],
                                lhs_sb[0][:, (n_rt - 1) * P : n_rt * P],
                                rhs_sb[0][:, 0:512],
                                start=True,
                                stop=True,
                            )
                    for rt in range(rt_lo, rt_hi):
                        exp_step(rt, split=(final_rep and last_batch and rt == n_rt - 1))
                        if final_rep and last_batch and rt == n_rt - 2:
                            # rt6 strips run under rt7's exp quarters
                            acc_step(rt)
                        if final_rep and last_batch and rt == n_rt - 1:
                            # tail: strip the final row-tile per exp-quarter
                            for q in range(4):
                                strips(rt, range(4 * q, 4 * q + 4))
                            del w_tiles[rt]
                    tc.no_sync_barrier()

                if final_rep:
                    emit_outputs(rowsum)

    nc.compile()
    return nc


def host_prep(cond_feature, sol_feature, temperature, n_rt=8, n_groups=4):
    """Build per-core input maps + host-side scalars."""
    import ml_dtypes

    c = np.asarray(cond_feature, dtype=np.float32).reshape(-1, D)
    s = np.asarray(sol_feature, dtype=np.float32).reshape(-1, D)
    n = c.shape[0]
    rows = P * n_rt
    cols = 2048 * n_groups

    a = float(np.exp(np.float64(np.asarray(temperature))))
    c2 = np.sum(c.astype(np.float64) ** 2, axis=1)
    s2 = np.sum(s.astype(np.float64) ** 2, axis=1)
    d2_mean = float(np.mean(c2) + np.mean(s2))
    B = a * float(np.sqrt(max(d2_mean, 1e-6)))

    lhsT = np.ascontiguousarray(-2.0 * c.T).astype(ml_dtypes.bfloat16)  # [D, n]
    rhsT = np.ascontiguousarray(s.T)[:, :cols].astype(ml_dtypes.bfloat16)
    s2b = np.broadcast_to(
        s2[:cols].astype(np.float16).reshape(1, cols), (P, cols)
    ).copy()
    cst = np.empty((P, 2), dtype=np.float32)
    cst[:, 0] = -a
    cst[:, 1] = B

    in_maps = []
    ncores = max(1, n // rows)
    for k in range(ncores):
        c2_k = (
            c2[k * rows : (k + 1) * rows]
            .astype(np.float32)
            .reshape(n_rt, P)
            .T.copy()
        )
        in_maps.append(
            {
                "lhsT": np.ascontiguousarray(lhsT[:, k * rows : (k + 1) * rows]),
                "rhsT": rhsT,
                "s2b": s2b,
                "c2m": c2_k,
                "cst": cst,
            }
        )

    # diagonal of sim in float64 (tiny O(N*D) host cost)
    dd = np.sqrt(np.maximum(np.sum((c.astype(np.float64) - s.astype(np.float64)) ** 2, axis=1), 0.0))
    sim_diag = -a * dd
    return in_maps, a, B, sim_diag


def host_post(results, B, sim_diag, n_rt=8, n_groups=4):
    """Combine per-core rowsums/colsums into the scalar loss."""
    lse_rows = []
    col_total = None
    for res in results:
        rs = np.asarray(res["rowsums"], dtype=np.float64)  # [P, n_rt+3]
        full = rs[:, : n_rt - 1]  # row-tiles 0..n_rt-2
        last = rs[:, n_rt - 1 :].sum(axis=1, keepdims=True)  # 4 quarter sums
        rs = np.concatenate([full, last], axis=1)  # [P, n_rt]
        # row order within core: rt*128 + p
        lse_rows.append(np.log(rs.T.reshape(-1)) - B)  # [rows]
        # colsums [4, 2048]: [m][b][j] -> column 2048*b + 512*m + j
        cs4 = np.asarray(res["colsums"], dtype=np.float64).reshape(4, 4, 512)
        cs = cs4.transpose(1, 0, 2).reshape(-1)
        col_total = cs if col_total is None else col_total + cs
    lse_row = np.concatenate(lse_rows)
    lse_col = np.log(col_total) - B

    loss_row = np.mean(lse_row - sim_diag[: lse_row.shape[0]])
    loss_col = np.mean(lse_col - sim_diag[: lse_col.shape[0]])
    return np.float32(0.5 * (loss_row + loss_col))


_NC_CACHE = {}


def _get_nc(n_rt=8, n_groups=4):
    key = (n_rt, n_groups)
    if key not in _NC_CACHE:
        _NC_CACHE[key] = build(n_rt, n_groups)
    return _NC_CACHE[key]


def run(cond_feature, sol_feature, temperature, trace=False):
    nc = _get_nc()
    in_maps, a, B, sim_diag = host_prep(cond_feature, sol_feature, temperature)
    res = run_bass_kernel_spmd(
        nc, in_maps, core_ids=list(range(NCORES)), trace=trace
    )
    loss = host_post(res.results, B, sim_diag)
    return loss, res


def kernel(cond_feature, sol_feature, temperature):
    loss, _ = run(cond_feature, sol_feature, temperature, trace=False)
    return loss
